# revision 24
# baseline (speedup 1.0000x reference)
"""Trainium2 Bass kernel for nn_AttnDBGNNLayer (8-core SPMD).

kernel(**inputs) takes the FULL inputs (as produced by setup_inputs) and
returns the FULL output (new_A, new_B), distributing across 8 NeuronCores.

Design:
- q-rows of both attentions sharded 8-way (1024 rows/core); K/V/Q computed
  from a feature-major fp8 x0^T streamed in 4 chunks with A/B-interleaved
  projections so the softmax pipeline starts early; single-pass unnormalized
  softmax (softmax is invariant to the k-bias so it is dropped); the
  out-projection is folded into V (Wvo = Wout @ Wv); K/Q/V stored fp8.
- exp(scores) emitted in fp8; attn@V and the softmax row-sum (ones-lhsT)
  run as fp8 DoubleRow matmuls accumulating in PSUM — no DVE accumulation.
  reciprocal_approx_fast + gpsimd partition_broadcast give a per-column
  scale, so normalization is one elementwise multiply (no PE transposes).
- normalized h (x64, fp8) of BOTH types is AllGathered once per row-half;
  each core reloads all-cores h^T with four contiguous DMAs into one
  combined tile and computes per-128-src-block tables on chip
  (tab = h @ wl^T scaled to x128 total, stored fp8), software-pipelined
  one step ahead of the aggregation so the fp8 copy latency hides.
- message aggregation as dense count-matrix DoubleRow matmuls:
  out^T += tab_pair^T x ct_pair with ct the per-core [8192 src, 1024 dst]
  edge-count matrix scaled by 2^-7 (exact in fp8 for counts <= 15), rows
  permuted to the gathered-h block order and swizzled for paired streaming;
  ct chunks are prefetched deep (12 dedicated buffers plus the freed
  x0/kt/vt slots) and split 4-way across DMA queues. lin_r folds in as a
  bf16 matmul; bias/degree corrections fold in as bf16 hi+lo split K=1
  matmuls (exact degrees, f32-accurate constants) in the same PSUM
  accumulation group, emitted after the first src-core so tables start
  immediately.
- outputs are produced feature-major and transposed on the host.
"""
import sys

if "/opt/trn_rl_repo" not in sys.path:
    sys.path.insert(0, "/opt/trn_rl_repo")

import numpy as np
import ml_dtypes

import concourse.bacc as bacc
import concourse.tile as tile
import concourse.mybir as mybir
from concourse import bass_utils

BF16 = ml_dtypes.bfloat16
FP8 = ml_dtypes.float8_e4m3

N = 8192
D = 128
NCORES = 8
R = N // NCORES       # 1024 rows per core
QG = 512              # q-group width (one AllGather half)
KB = N // 128         # 64 k-blocks
SCALE = 1.0 / np.sqrt(np.float32(D))
TAB_SCALE = 128.0     # total table scale; counts scaled by 2^-7
H_SCALE = 64.0        # gathered h stored as h*64 fp8
WL_SCALE = TAB_SCALE / H_SCALE

F32 = mybir.dt.float32
BF = mybir.dt.bfloat16
F8 = mybir.dt.float8e4
DR = mybir.MatmulPerfMode.DoubleRow

G = ("AB", "BA", "AA")
GI = {g: i for i, g in enumerate(G)}
SRC_T = {"AB": "A", "BA": "B", "AA": "A"}
DST_T = {"AB": "B", "BA": "A", "AA": "A"}

# bf16 weight blob layout: [128,128] slices (wlT_AB|wlT_AA adjacent)
WB_ORDER = ["wqT_A", "wkT_A", "wvoT_A", "wqT_B", "wkT_B", "wvoT_B",
            "wlT_AB", "wlT_AA", "wlT_BA", "wrT_A", "wrT_B"]
# f32 col blob: [128, 2]
CB_ORDER = ["bqs_A", "bqs_B"]
# bf16 row blob: hi/lo split constants then degrees
RB_ORDER = ["c0h_A", "c0l_A", "c0h_B", "c0l_B",
            "c1h_AB", "c1l_AB", "c1h_BA", "c1l_BA", "c1h_AA", "c1l_AA"]
RB_LEN = 128 * len(RB_ORDER) + 3 * R

_PROG_CACHE = {}


def build_program(dbg=False, stage=3):
    nc = bacc.Bacc("TRN2", target_bir_lowering=False, debug=False,
                   num_devices=NCORES)

    x0t = {t: nc.dram_tensor(f"x0t_{t}", [128, N], F8, kind="ExternalInput")
           for t in "AB"}
    x0q = {t: nc.dram_tensor(f"x0q_{t}", [128, R], F8, kind="ExternalInput")
           for t in "AB"}
    wblob = nc.dram_tensor("wblob", [128, 128 * len(WB_ORDER)], BF,
                           kind="ExternalInput")
    cblob = nc.dram_tensor("cblob", [128, len(CB_ORDER)], F32,
                           kind="ExternalInput")
    rblob = nc.dram_tensor("rblob", [1, RB_LEN], BF, kind="ExternalInput")
    ct = {g: nc.dram_tensor(f"ct_{g}", [1024, 8 * R], F8,
                            kind="ExternalInput") for g in G}
    out_d = {t: nc.dram_tensor(f"out_{t}", [128, R], F32,
                               kind="ExternalOutput") for t in "AB"}
    dbg_d = {}
    if dbg:
        for t in "AB":
            dbg_d[f"ht_{t}"] = nc.dram_tensor(f"dbg_ht_{t}", [128, R], BF,
                                              kind="ExternalOutput")

    h_loc = nc.dram_tensor("h_loc", [256, 2 * QG], F8)
    h_sh = nc.dram_tensor("h_sh", [2048, 2 * QG], F8, addr_space="Shared")

    with tile.TileContext(nc) as tc:
        with (
            tc.tile_pool(name="const", bufs=1) as cp,
            tc.tile_pool(name="big", bufs=1) as bp,
            tc.tile_pool(name="pt", bufs=4) as ptp,
            tc.tile_pool(name="ctp", bufs=12) as ctp,
            tc.tile_pool(name="tabp", bufs=3) as tabp,
            tc.tile_pool(name="ps_big", bufs=2, space="PSUM") as ps_big,
            tc.tile_pool(name="ps_ut", bufs=2, space="PSUM") as ps_ut,
            tc.tile_pool(name="ps_rs", bufs=2, space="PSUM") as ps_rs,
        ):
            # ---------------- inputs (weights first, x0 in 4 chunks)
            wb = cp.tile([128, 128 * len(WB_ORDER)], BF, tag="wb")
            nc.sync.dma_start(out=wb[:], in_=wblob[:])
            W = {k: wb[:, i * 128:(i + 1) * 128]
                 for i, k in enumerate(WB_ORDER)}
            wl_cat_A = wb[:, 6 * 128:8 * 128]  # [wlT_AB | wlT_AA]
            x0q_s = {}
            for t in "AB":
                x0q_s[t] = bp.tile([128, R], F8, tag=f"x0q_{t}",
                                   name=f"x0q_{t}")
                nc.sync.dma_start(out=x0q_s[t][:], in_=x0q[t][:])
            cb = cp.tile([128, len(CB_ORDER)], F32, tag="cb")
            nc.sync.dma_start(out=cb[:], in_=cblob[:])
            C = {k: cb[:, i:i + 1] for i, k in enumerate(CB_ORDER)}
            rb = cp.tile([1, RB_LEN], BF, tag="rb")
            nc.sync.dma_start(out=rb[:], in_=rblob[:])
            RW = {k: rb[:, i * 128:(i + 1) * 128]
                  for i, k in enumerate(RB_ORDER)}
            DEG = {g: rb[:, 1280 + GI[g] * R: 1280 + (GI[g] + 1) * R]
                   for g in G}

            x0_s = {}
            for t in "AB":
                x0_s[t] = bp.tile([128, N], F8, tag=f"x0t_{t}",
                                  name=f"x0_{t}")
            for ch in range(4):
                for t in "AB":
                    sl = slice(ch * 2048, (ch + 1) * 2048)
                    nc.sync.dma_start(out=x0_s[t][:, sl], in_=x0t[t][:, sl])

            # DoubleRow lhsT needs the k-pair dim stride %16 == 0
            ones2 = cp.tile([128, 32], F8, tag="ones2")
            nc.vector.memset(ones2[:], 1.0)
            ones_row = cp.tile([1, 512], BF, tag="ones_row")
            nc.vector.memset(ones_row[:], 1.0)

            ht = {t: bp.tile([128, R], BF, tag=f"ht_{t}", name=f"ht_{t}")
                  for t in "AB"}

            # ---------------- QKV, A/B and chunk interleaved
            kt = {}
            vt = {}
            qt = {}
            for t in "AB":
                kt[t] = bp.tile([128, N], F8, tag=f"kt_{t}", name=f"kt_{t}")
                vt[t] = bp.tile([128, N], F8, tag=f"vt_{t}", name=f"vt_{t}")
                qt[t] = bp.tile([128, R], F8, tag=f"qt_{t}", name=f"qt_{t}")
                ps_q = ps_big.tile([128, 1024], F32, tag="sc",
                                   name=f"psq_{t}")
                for j in range(2):
                    nc.tensor.matmul(ps_q[:, j * 512:(j + 1) * 512],
                                     lhsT=W[f"wqT_{t}"],
                                     rhs=x0q_s[t][:, j * 512:(j + 1) * 512],
                                     start=True, stop=True)
                nc.vector.tensor_scalar(
                    qt[t][:], ps_q[:], float(SCALE), C[f"bqs_{t}"],
                    op0=mybir.AluOpType.mult, op1=mybir.AluOpType.add)
            def emit_k(t, j4, on_act):
                ps_k = ps_big.tile([128, 1024], F32, tag="sc",
                                   name=f"psk_{t}_{j4}")
                for j in range(2):
                    nc.tensor.matmul(
                        ps_k[:, j * 512:(j + 1) * 512],
                        lhsT=W[f"wkT_{t}"],
                        rhs=x0_s[t][:, j4 * 1024 + j * 512:
                                    j4 * 1024 + (j + 1) * 512],
                        start=True, stop=True)
                dst = kt[t][:, j4 * 1024:(j4 + 1) * 1024]
                if on_act:
                    nc.scalar.activation(dst, ps_k[:],
                                         mybir.ActivationFunctionType.Copy)
                else:
                    nc.vector.tensor_copy(dst, ps_k[:])

            def emit_v(t, vg, on_act):
                ps_v = ps_big.tile([128, 512], F32, tag="sc",
                                   name=f"psv_{t}_{vg}")
                for i in range(4):
                    nb = vg * 4 + i
                    nc.tensor.matmul(
                        ps_v[:, i * 128:(i + 1) * 128],
                        lhsT=x0_s[t][:, nb * 128:(nb + 1) * 128],
                        rhs=W[f"wvoT_{t}"],
                        start=True, stop=True)
                dst = vt[t][:, vg * 512:(vg + 1) * 512]
                if on_act:
                    nc.scalar.activation(dst, ps_v[:],
                                         mybir.ActivationFunctionType.Copy)
                else:
                    nc.vector.tensor_copy(dst, ps_v[:])

            # chunk 0 upfront (ACT/DVE alternating); chunks 1-3 are emitted
            # inside the qg0 attention loop (DVE copies; ACT is doing exp)
            for t in "AB":
                emit_k(t, 0, True)
                emit_k(t, 1, False)
            for t in "AB":
                for vg in range(4):
                    emit_v(t, vg, vg % 2 == 0)
            qkv_sched = {}
            for ch in (1, 2, 3):
                pieces = ([("k", t, j4) for t in "AB"
                           for j4 in (2 * ch, 2 * ch + 1)]
                          + [("v", t, vg) for t in "AB"
                             for vg in range(4 * ch, 4 * ch + 4)])
                for idx, p in enumerate(pieces):
                    qkv_sched.setdefault(8 * (ch - 1) + 1 + idx // 2,
                                         []).append(p)

            # ---------------- attention, A/B interleaved; gather per half
            ones2_3 = ones2[:].rearrange("p (j o) -> p j o", j=2)[:, :, 0:1]
            for qg in range(2):
                q_sl = slice(qg * QG, (qg + 1) * QG)
                ut_ps = {}
                rs_ps = {}
                for t in "AB":
                    ut_ps[t] = ps_ut.tile([128, QG], F32, tag="ut",
                                          name=f"utps_{t}_{qg}")
                    rs_ps[t] = ps_rs.tile([1, QG], F32, tag="rs",
                                          name=f"rsps_{t}_{qg}")

                def rs_ap(t, qg):
                    return rs_ps[t][:]
                for pr in range(KB // 2):
                    kb0 = 2 * pr
                    if qg == 0:
                        for kind, tt, idx in qkv_sched.get(pr, []):
                            if kind == "k":
                                emit_k(tt, idx, False)
                            else:
                                emit_v(tt, idx, False)
                    for t in "AB":
                        sc = ps_big.tile([128, 1024], F32, tag="sc",
                                         name=f"sc_{t}_{qg}_{pr}")
                        nc.tensor.matmul(sc[:, :512],
                                         lhsT=kt[t][:, kb0 * 128:(kb0 + 1) * 128],
                                         rhs=qt[t][:, q_sl],
                                         start=True, stop=True)
                        nc.tensor.matmul(sc[:, 512:],
                                         lhsT=kt[t][:, (kb0 + 1) * 128:(kb0 + 2) * 128],
                                         rhs=qt[t][:, q_sl],
                                         start=True, stop=True)
                        pt = ptp.tile([128, 1024], F8, tag="pt",
                                      name=f"pt_{t}_{qg}_{pr}")
                        nc.scalar.activation(pt[:], sc[:],
                                             mybir.ActivationFunctionType.Exp)
                        pt3 = pt[:].rearrange("p (j q) -> p j q", j=2)
                        v3 = vt[t][:, kb0 * 128:(kb0 + 2) * 128].rearrange(
                            "p (j f) -> p j f", j=2)
                        nc.tensor.matmul(ut_ps[t][:], lhsT=v3, rhs=pt3,
                                         start=(pr == 0),
                                         stop=(pr == KB // 2 - 1),
                                         perf_mode=DR)
                        nc.tensor.matmul(rs_ap(t, qg), lhsT=ones2_3, rhs=pt3,
                                         start=(pr == 0),
                                         stop=(pr == KB // 2 - 1),
                                         perf_mode=DR)

                # normalize: ht = ut * broadcast(1/rowsum)
                for ti, t in enumerate("AB"):
                    rv = bp.tile([1, QG], F32, tag=f"rv_{t}",
                                 name=f"rv_{t}_{qg}")
                    nc.vector.reciprocal_approx_fast(rv[:], rs_ap(t, qg))
                    rbc = bp.tile([128, QG], F32, tag=f"rbc_{t}",
                                  name=f"rbc_{t}_{qg}")
                    nc.gpsimd.partition_broadcast(rbc[:], rv[:])
                    nc.vector.tensor_tensor(ht[t][:, q_sl], ut_ps[t][:],
                                            rbc[:], op=mybir.AluOpType.mult)
                    h8 = bp.tile([128, QG], F8, tag=f"h8_{t}",
                                 name=f"h8_{t}_{qg}")
                    nc.gpsimd.tensor_scalar(h8[:], ht[t][:, q_sl],
                                            float(H_SCALE), None,
                                            op0=mybir.AluOpType.mult)
                    nc.sync.dma_start(
                        out=h_loc[qg * 128:(qg + 1) * 128,
                                  ti * QG:(ti + 1) * QG],
                        in_=h8[:])
                nc.gpsimd.collective_compute(
                    "AllGather", mybir.AluOpType.bypass,
                    replica_groups=[list(range(NCORES))],
                    ins=[h_loc[qg * 128:(qg + 1) * 128, :]],
                    outs=[h_sh[qg * 1024:(qg + 1) * 1024, :]])

            if dbg:
                for t in "AB":
                    nc.sync.dma_start(out=dbg_d[f"ht_{t}"][:], in_=ht[t][:])

            # ---------------- phase 2: dense count-matrix aggregation
            if stage >= 2:
                hTall = bp.tile([128, 2 * N], F8, tag="hTall",
                                name="hTall")
                for h in range(2):
                    for q in range(4):  # split over cores -> 4 queues
                        nc.sync.dma_start(
                            out=hTall[:, h * 8192 + q * 2048:
                                      h * 8192 + (q + 1) * 2048
                                      ].rearrange("f (c u) -> f c u", c=2),
                            in_=h_sh[h * 1024 + q * 256:
                                     h * 1024 + (q + 1) * 256, :
                                     ].rearrange("(c f) u -> f c u", f=128))

                def hT_blk(t, b):
                    # virtual src-block b -> hTall 128-col block for type t
                    hh, cc, ww = b >> 5, (b >> 2) & 7, b & 3
                    j = hh * 64 + cc * 8 + (0 if t == "A" else 4) + ww
                    return hTall[:, j * 128:(j + 1) * 128]

                po = {}
                for t in "AB":
                    po[t] = ps_big.tile([128, 1024], F32, tag="sc",
                                        name=f"po_{t}")

                def emit_fixups():
                    for t in "AB":
                        for h in range(2):
                            h_sl = slice(h * 512, (h + 1) * 512)
                            nc.tensor.matmul(po[t][:, h_sl],
                                             lhsT=W[f"wrT_{t}"],
                                             rhs=ht[t][:, h_sl],
                                             start=False, stop=False)
                            for part in ("h", "l"):
                                nc.tensor.matmul(po[t][:, h_sl],
                                                 lhsT=RW[f"c0{part}_{t}"],
                                                 rhs=ones_row[:],
                                                 start=False, stop=False)
                    for g in G:
                        for h in range(2):
                            for part in ("h", "l"):
                                nc.tensor.matmul(
                                    po[DST_T[g]][:, h * 512:(h + 1) * 512],
                                    lhsT=RW[f"c1{part}_{g}"],
                                    rhs=DEG[g][:, h * 512:(h + 1) * 512],
                                    start=False, stop=False)

                # table computation is software-pipelined one sbp ahead of
                # the aggregation matmuls so the fp8 copy latency hides;
                # tpB borrows the idle ps_rs slots.
                def emit_tab(i):
                    scg, sbp = divmod(i, 4)
                    j0 = scg * 8 + sbp * 2
                    tpA = ps_ut.tile([128, 512], F32, tag="ut",
                                     name=f"tpA_{scg}_{sbp}")
                    tpB = ps_rs.tile([128, 512], F32, tag="rs",
                                     name=f"tpB_{scg}_{sbp}")
                    for j in range(2):
                        nc.tensor.matmul(tpA[:, j * 256:(j + 1) * 256],
                                         lhsT=hT_blk("A", j0 + j),
                                         rhs=wl_cat_A,
                                         start=True, stop=True)
                        nc.tensor.matmul(tpB[:, j * 128:(j + 1) * 128],
                                         lhsT=hT_blk("B", j0 + j),
                                         rhs=W["wlT_BA"],
                                         start=True, stop=True)
                    tabA = tabp.tile([128, 512], F8, tag="tabA",
                                     name=f"tabA_{scg}_{sbp}")
                    nc.scalar.activation(tabA[:], tpA[:],
                                         mybir.ActivationFunctionType.Copy)
                    tabB = tabp.tile([128, 256], F8, tag="tabB",
                                     name=f"tabB_{scg}_{sbp}")
                    nc.vector.tensor_copy(tabB[:], tpB[:, :256])
                    tabA3 = tabA[:].rearrange("p (j f) -> p j f", j=2)
                    tabB3 = tabB[:].rearrange("p (j f) -> p j f", j=2)
                    return {"AB": tabA3[:, :, 0:128],
                            "AA": tabA3[:, :, 128:256],
                            "BA": tabB3}

                # ct chunks ride freed x0/kt/vt slots past the ctp pool.
                # kt/vt free only when attention ends, so their chunks go
                # last in consumption order; x0 frees right after QKV.
                spill = {12: ("x0t_A", bp), 13: ("x0t_B", bp),
                         20: ("kt_A", bp), 21: ("kt_B", bp),
                         22: ("vt_A", bp), 23: ("vt_B", bp)}
                tabs = {0: emit_tab(0)}
                started = set()
                for scg in range(8):
                    ct_s = {}
                    for g in G:
                        k = scg * 3 + GI[g]
                        if k in spill:
                            tag, pool = spill[k]
                            ct_s[g] = pool.tile([128, 8 * R], F8, tag=tag,
                                                name=f"ct_{g}_{scg}")
                        else:
                            ct_s[g] = ctp.tile([128, 8 * R], F8, tag="ct",
                                               name=f"ct_{g}_{scg}")
                        for q in range(4):  # split -> 4 parallel queues
                            nc.sync.dma_start(
                                out=ct_s[g][:, q * 2048:(q + 1) * 2048],
                                in_=ct[g][scg * 128:(scg + 1) * 128,
                                          q * 2048:(q + 1) * 2048])
                    for sbp in range(4):
                        i = scg * 4 + sbp
                        if i + 1 < 32:
                            tabs[i + 1] = emit_tab(i + 1)
                        lhsT_of = tabs.pop(i)
                        last = (scg == 7 and sbp == 3)
                        for g in G:
                            for h in range(2):
                                rhs = ct_s[g][:, sbp * 2048 + h * 1024:
                                              sbp * 2048 + (h + 1) * 1024
                                              ].rearrange(
                                                  "p (j d) -> p j d", j=2)
                                is_stop = last and (
                                    g == ("AB" if DST_T[g] == "B" else "AA"))
                                key = (DST_T[g], h)
                                nc.tensor.matmul(
                                    po[DST_T[g]][:, h * 512:(h + 1) * 512],
                                    lhsT=lhsT_of[g], rhs=rhs,
                                    start=(key not in started),
                                    stop=is_stop, perf_mode=DR)
                                started.add(key)
                    if scg == 0:
                        emit_fixups()

                for t in "AB":
                    for h in range(2):
                        osb = bp.tile([128, 512], F32, tag=f"osb_{t}{h}",
                                      name=f"osb_{t}_{h}")
                        nc.vector.tensor_copy(osb[:],
                                              po[t][:, h * 512:(h + 1) * 512])
                        nc.sync.dma_start(
                            out=out_d[t][:, h * 512:(h + 1) * 512],
                            in_=osb[:])

    nc.compile()
    return nc


# ---------------------------------------------------------------- host prep

def _pos_of_src():
    """global node id -> ct row position (virtual block * 128 + s)."""
    src = np.arange(N)
    c = src >> 10
    rr = src & 1023
    half = rr >> 9
    w = rr & 511
    wsub = w >> 7
    s = w & 127
    return (half * 32 + c * 4 + wsub) * 128 + s


def _hl(x):
    """split f32 vector into bf16 (hi, lo)."""
    hi = x.astype(BF16)
    lo = (x - hi.astype(np.float32)).astype(BF16)
    return hi, lo


def _prep(inputs, dbg=False):
    ins = {k: np.asarray(v) for k, v in inputs.items()}

    def bf(x):
        return np.ascontiguousarray(np.asarray(x, np.float32)).astype(BF16)

    com = {}
    for t in "AB":
        iw = ins[f"inW_{t}"].astype(np.float32)
        ib = ins[f"inB_{t}"].astype(np.float32)
        ow = ins[f"outW_{t}"].astype(np.float32)
        ob = ins[f"outB_{t}"].astype(np.float32)
        com[f"wqT_{t}"] = iw[0:128].T
        com[f"wkT_{t}"] = iw[128:256].T
        com[f"wvoT_{t}"] = (ow @ iw[256:384]).T
        com[f"bqs_{t}"] = ib[0:128] * SCALE
        com[f"bout_eff_{t}"] = ow @ ib[256:384] + ob
    c1 = {}
    for g in G:
        com[f"wlT_{g}"] = ins[f"wl_{g}"].astype(np.float32).T * WL_SCALE
        c1[g] = (ins[f"wl_{g}"].astype(np.float32)
                 @ com[f"bout_eff_{SRC_T[g]}"])
    com["wrT_B"] = ins["wr_AB"].astype(np.float32).T
    com["wrT_A"] = (ins["wr_BA"] + ins["wr_AA"]).astype(np.float32).T
    c0 = {
        "B": (ins["bl_AB"].astype(np.float32)
              + ins["wr_AB"].astype(np.float32) @ com["bout_eff_B"]),
        "A": (ins["bl_BA"].astype(np.float32)
              + ins["bl_AA"].astype(np.float32)
              + (ins["wr_BA"] + ins["wr_AA"]).astype(np.float32)
              @ com["bout_eff_A"]),
    }
    rbc = {}
    for t in "AB":
        rbc[f"c0h_{t}"], rbc[f"c0l_{t}"] = _hl(c0[t])
    for g in G:
        rbc[f"c1h_{g}"], rbc[f"c1l_{g}"] = _hl(c1[g])

    wblob = bf(np.concatenate([com[k] for k in WB_ORDER], axis=1))
    cblob = np.stack([com[f"bqs_{t}"] for t in "AB"],
                     axis=1).astype(np.float32)

    x0T = {t: np.ascontiguousarray(
        ins[f"x_{t}"][:, 0, :].astype(np.float32).T).astype(FP8)
        for t in "AB"}

    pos_of = _pos_of_src()
    cts = {}
    degs = {}
    for g in G:
        src = np.asarray(ins[f"ei_{g}"][0], np.int64)
        dst = np.asarray(ins[f"ei_{g}"][1], np.int64)
        per_core = []
        dgs = []
        for c in range(NCORES):
            sel = (dst >> 10) == c
            s_c = pos_of[src[sel]]
            d_c = dst[sel] - c * R
            cmat = np.zeros((N, R), np.float32)
            np.add.at(cmat, (s_c, d_c), 1.0)
            cmat *= 1.0 / TAB_SCALE
            swz = np.ascontiguousarray(
                cmat.reshape(8, 4, 2, 128, 2, 512).transpose(0, 3, 1, 4, 2, 5)
                .reshape(1024, 8 * R))
            per_core.append(swz.astype(FP8))
            dgs.append(np.bincount(d_c, minlength=R).astype(BF16))
        cts[g] = per_core
        degs[g] = dgs

    in_maps = []
    for c in range(NCORES):
        rblob = np.concatenate(
            [rbc[k].astype(BF16) for k in RB_ORDER]
            + [degs[g][c] for g in G]).reshape(1, -1)
        m = {"wblob": wblob, "cblob": cblob, "rblob": rblob}
        for t in "AB":
            m[f"x0t_{t}"] = x0T[t]
            m[f"x0q_{t}"] = np.ascontiguousarray(x0T[t][:, c * R:(c + 1) * R])
        for g in G:
            m[f"ct_{g}"] = cts[g][c]
        in_maps.append(m)
    return in_maps


def kernel(**inputs):
    in_maps = _prep(inputs)
    if "prog" not in _PROG_CACHE:
        _PROG_CACHE["prog"] = build_program()
    nc = _PROG_CACHE["prog"]
    res = bass_utils.run_bass_kernel_spmd(
        nc, in_maps, core_ids=list(range(NCORES)))
    x_A = np.asarray(inputs["x_A"], np.float32)
    x_B = np.asarray(inputs["x_B"], np.float32)
    new_A = x_A.copy()
    new_B = x_B.copy()
    for c in range(NCORES):
        new_A[c * R:(c + 1) * R, 0, :] = res.results[c]["out_A"].T
        new_B[c * R:(c + 1) * R, 0, :] = res.results[c]["out_B"].T
    return new_A, new_B


# revision 25
# speedup vs baseline: 1.0650x; 1.0650x over previous
"""Trainium2 Bass kernel for nn_AttnDBGNNLayer (8-core SPMD).

kernel(**inputs) takes the FULL inputs (as produced by setup_inputs) and
returns the FULL output (new_A, new_B), distributing across 8 NeuronCores.

Design:
- q-rows of both attentions sharded 8-way (1024 rows/core); K/V/Q computed
  from a feature-major fp8 x0^T streamed in 4 chunks with A/B-interleaved
  projections so the softmax pipeline starts early; single-pass unnormalized
  softmax (softmax is invariant to the k-bias so it is dropped); the
  out-projection is folded into V (Wvo = Wout @ Wv); K/Q/V stored fp8.
- exp(scores) emitted in fp8; attn@V and the softmax row-sum (ones-lhsT)
  run as fp8 DoubleRow matmuls accumulating in PSUM — no DVE accumulation.
  reciprocal_approx_fast + gpsimd partition_broadcast give a per-column
  scale, so normalization is one elementwise multiply (no PE transposes).
- normalized h (x64, fp8) of BOTH types is AllGathered once per row-half;
  each core reloads all-cores h^T with four contiguous DMAs into one
  combined tile and computes per-128-src-block tables on chip
  (tab = h @ wl^T scaled to x128 total, stored fp8), software-pipelined
  one step ahead of the aggregation so the fp8 copy latency hides.
- message aggregation as dense count-matrix DoubleRow matmuls:
  out^T += tab_pair^T x ct_pair with ct the per-core [8192 src, 1024 dst]
  edge-count matrix scaled by 2^-7 (exact in fp8 for counts <= 15), rows
  permuted to the gathered-h block order and swizzled for paired streaming;
  ct chunks are prefetched deep (12 dedicated buffers plus the freed
  x0/kt/vt slots) and split 4-way across DMA queues. lin_r folds in as a
  bf16 matmul; bias/degree corrections fold in as bf16 hi+lo split K=1
  matmuls (exact degrees, f32-accurate constants) in the same PSUM
  accumulation group, emitted after the first src-core so tables start
  immediately.
- outputs are produced feature-major and transposed on the host.
"""
import sys

if "/opt/trn_rl_repo" not in sys.path:
    sys.path.insert(0, "/opt/trn_rl_repo")

import numpy as np
import ml_dtypes

import concourse.bacc as bacc
import concourse.tile as tile
import concourse.mybir as mybir
from concourse import bass_utils

BF16 = ml_dtypes.bfloat16
FP8 = ml_dtypes.float8_e4m3

N = 8192
D = 128
NCORES = 8
R = N // NCORES       # 1024 rows per core
QG = 512              # q-group width (one AllGather half)
KB = N // 128         # 64 k-blocks
SCALE = 1.0 / np.sqrt(np.float32(D))
TAB_SCALE = 128.0     # total table scale; counts scaled by 2^-7
H_SCALE = 64.0        # gathered h stored as h*64 fp8
WL_SCALE = TAB_SCALE / H_SCALE

F32 = mybir.dt.float32
BF = mybir.dt.bfloat16
F8 = mybir.dt.float8e4
DR = mybir.MatmulPerfMode.DoubleRow

G = ("AB", "BA", "AA")
GI = {g: i for i, g in enumerate(G)}
SRC_T = {"AB": "A", "BA": "B", "AA": "A"}
DST_T = {"AB": "B", "BA": "A", "AA": "A"}

# bf16 weight blob layout: [128,128] slices (wlT_AB|wlT_AA adjacent)
WB_ORDER = ["wqT_A", "wkT_A", "wvoT_A", "wqT_B", "wkT_B", "wvoT_B",
            "wlT_AB", "wlT_AA", "wlT_BA", "wrT_A", "wrT_B"]
# f32 col blob: [128, 2]
CB_ORDER = ["bqs_A", "bqs_B"]
# bf16 row blob: hi/lo split constants then degrees
RB_ORDER = ["c0h_A", "c0l_A", "c0h_B", "c0l_B",
            "c1h_AB", "c1l_AB", "c1h_BA", "c1l_BA", "c1h_AA", "c1l_AA"]
RB_LEN = 128 * len(RB_ORDER) + 3 * R

_PROG_CACHE = {}


def build_program(dbg=False, stage=3):
    nc = bacc.Bacc("TRN2", target_bir_lowering=False, debug=False,
                   num_devices=NCORES)

    x0t = {t: nc.dram_tensor(f"x0t_{t}", [128, N], F8, kind="ExternalInput")
           for t in "AB"}
    x0q = {t: nc.dram_tensor(f"x0q_{t}", [128, R], F8, kind="ExternalInput")
           for t in "AB"}
    wblob = nc.dram_tensor("wblob", [128, 128 * len(WB_ORDER)], BF,
                           kind="ExternalInput")
    cblob = nc.dram_tensor("cblob", [128, len(CB_ORDER)], F32,
                           kind="ExternalInput")
    rblob = nc.dram_tensor("rblob", [1, RB_LEN], BF, kind="ExternalInput")
    ct = {g: nc.dram_tensor(f"ct_{g}", [1024, 8 * R], F8,
                            kind="ExternalInput") for g in G}
    out_d = {t: nc.dram_tensor(f"out_{t}", [128, R], F32,
                               kind="ExternalOutput") for t in "AB"}
    dbg_d = {}
    if dbg:
        for t in "AB":
            dbg_d[f"ht_{t}"] = nc.dram_tensor(f"dbg_ht_{t}", [128, R], BF,
                                              kind="ExternalOutput")

    h_loc = nc.dram_tensor("h_loc", [256, 2 * QG], F8)
    h_sh = nc.dram_tensor("h_sh", [2048, 2 * QG], F8, addr_space="Shared")

    with tile.TileContext(nc) as tc:
        with (
            tc.tile_pool(name="const", bufs=1) as cp,
            tc.tile_pool(name="big", bufs=1) as bp,
            tc.tile_pool(name="pt", bufs=4) as ptp,
            tc.tile_pool(name="ctp", bufs=12) as ctp,
            tc.tile_pool(name="tabp", bufs=3) as tabp,
            tc.tile_pool(name="ps_big", bufs=2, space="PSUM") as ps_big,
            tc.tile_pool(name="ps_ut", bufs=2, space="PSUM") as ps_ut,
            tc.tile_pool(name="ps_rs", bufs=2, space="PSUM") as ps_rs,
        ):
            # ---------------- inputs (weights first, x0 in 4 chunks)
            wb = cp.tile([128, 128 * len(WB_ORDER)], BF, tag="wb")
            nc.sync.dma_start(out=wb[:], in_=wblob[:])
            W = {k: wb[:, i * 128:(i + 1) * 128]
                 for i, k in enumerate(WB_ORDER)}
            wl_cat_A = wb[:, 6 * 128:8 * 128]  # [wlT_AB | wlT_AA]
            x0q_s = {}
            for t in "AB":
                x0q_s[t] = bp.tile([128, R], F8, tag=f"x0q_{t}",
                                   name=f"x0q_{t}")
                nc.sync.dma_start(out=x0q_s[t][:], in_=x0q[t][:])
            cb = cp.tile([128, len(CB_ORDER)], F32, tag="cb")
            nc.sync.dma_start(out=cb[:], in_=cblob[:])
            C = {k: cb[:, i:i + 1] for i, k in enumerate(CB_ORDER)}
            rb = cp.tile([1, RB_LEN], BF, tag="rb")
            nc.sync.dma_start(out=rb[:], in_=rblob[:])
            RW = {k: rb[:, i * 128:(i + 1) * 128]
                  for i, k in enumerate(RB_ORDER)}
            DEG = {g: rb[:, 1280 + GI[g] * R: 1280 + (GI[g] + 1) * R]
                   for g in G}

            x0_s = {}
            for t in "AB":
                x0_s[t] = bp.tile([128, N], F8, tag=f"x0t_{t}",
                                  name=f"x0_{t}")
            for ch in range(4):
                for t in "AB":
                    sl = slice(ch * 2048, (ch + 1) * 2048)
                    nc.sync.dma_start(out=x0_s[t][:, sl], in_=x0t[t][:, sl])

            # DoubleRow lhsT needs the k-pair dim stride %16 == 0
            ones2 = cp.tile([128, 32], F8, tag="ones2")
            nc.vector.memset(ones2[:], 1.0)
            ones_row = cp.tile([1, 512], BF, tag="ones_row")
            nc.vector.memset(ones_row[:], 1.0)

            ht = {t: bp.tile([128, R], BF, tag=f"ht_{t}", name=f"ht_{t}")
                  for t in "AB"}

            # ---------------- QKV, A/B and chunk interleaved
            kt = {}
            vt = {}
            qt = {}
            for t in "AB":
                kt[t] = bp.tile([128, N], F8, tag=f"kt_{t}", name=f"kt_{t}")
                vt[t] = bp.tile([128, N], F8, tag=f"vt_{t}", name=f"vt_{t}")
                qt[t] = bp.tile([128, R], F8, tag=f"qt_{t}", name=f"qt_{t}")
                ps_q = ps_big.tile([128, 1024], F32, tag="sc",
                                   name=f"psq_{t}")
                for j in range(2):
                    nc.tensor.matmul(ps_q[:, j * 512:(j + 1) * 512],
                                     lhsT=W[f"wqT_{t}"],
                                     rhs=x0q_s[t][:, j * 512:(j + 1) * 512],
                                     start=True, stop=True)
                nc.vector.tensor_scalar(
                    qt[t][:], ps_q[:], float(SCALE), C[f"bqs_{t}"],
                    op0=mybir.AluOpType.mult, op1=mybir.AluOpType.add)
            for ch in range(4):
                for t in "AB":
                    for j4 in (2 * ch, 2 * ch + 1):
                        ps_k = ps_big.tile([128, 1024], F32, tag="sc",
                                           name=f"psk_{t}_{j4}")
                        for j in range(2):
                            nc.tensor.matmul(
                                ps_k[:, j * 512:(j + 1) * 512],
                                lhsT=W[f"wkT_{t}"],
                                rhs=x0_s[t][:, j4 * 1024 + j * 512:
                                            j4 * 1024 + (j + 1) * 512],
                                start=True, stop=True)
                        dst = kt[t][:, j4 * 1024:(j4 + 1) * 1024]
                        if j4 % 2 == 0:
                            nc.scalar.activation(
                                dst, ps_k[:],
                                mybir.ActivationFunctionType.Copy)
                        else:
                            nc.vector.tensor_copy(dst, ps_k[:])
                for t in "AB":
                    for vg in range(4 * ch, 4 * ch + 4):
                        ps_v = ps_ut.tile([128, 512], F32, tag="ut",
                                          name=f"psv_{t}_{vg}")
                        for i in range(4):
                            nb = vg * 4 + i
                            nc.tensor.matmul(
                                ps_v[:, i * 128:(i + 1) * 128],
                                lhsT=x0_s[t][:, nb * 128:(nb + 1) * 128],
                                rhs=W[f"wvoT_{t}"],
                                start=True, stop=True)
                        dst = vt[t][:, vg * 512:(vg + 1) * 512]
                        if vg % 2 == 0:
                            nc.vector.tensor_copy(dst, ps_v[:])
                        else:
                            nc.scalar.activation(
                                dst, ps_v[:],
                                mybir.ActivationFunctionType.Copy)

            # ---------------- attention, A/B interleaved; gather per half
            ones2_3 = ones2[:].rearrange("p (j o) -> p j o", j=2)[:, :, 0:1]
            for qg in range(2):
                q_sl = slice(qg * QG, (qg + 1) * QG)
                ut_ps = {}
                rs_ps = {}
                for t in "AB":
                    ut_ps[t] = ps_ut.tile([128, QG], F32, tag="ut",
                                          name=f"utps_{t}_{qg}")
                    rs_ps[t] = ps_rs.tile([1, QG], F32, tag="rs",
                                          name=f"rsps_{t}_{qg}")

                def rs_ap(t, qg):
                    return rs_ps[t][:]
                for pr in range(KB // 2):
                    kb0 = 2 * pr
                    for t in "AB":
                        sc = ps_big.tile([128, 1024], F32, tag="sc",
                                         name=f"sc_{t}_{qg}_{pr}")
                        nc.tensor.matmul(sc[:, :512],
                                         lhsT=kt[t][:, kb0 * 128:(kb0 + 1) * 128],
                                         rhs=qt[t][:, q_sl],
                                         start=True, stop=True)
                        nc.tensor.matmul(sc[:, 512:],
                                         lhsT=kt[t][:, (kb0 + 1) * 128:(kb0 + 2) * 128],
                                         rhs=qt[t][:, q_sl],
                                         start=True, stop=True)
                        pt = ptp.tile([128, 1024], F8, tag="pt",
                                      name=f"pt_{t}_{qg}_{pr}")
                        nc.scalar.activation(pt[:], sc[:],
                                             mybir.ActivationFunctionType.Exp)
                        pt3 = pt[:].rearrange("p (j q) -> p j q", j=2)
                        v3 = vt[t][:, kb0 * 128:(kb0 + 2) * 128].rearrange(
                            "p (j f) -> p j f", j=2)
                        nc.tensor.matmul(ut_ps[t][:], lhsT=v3, rhs=pt3,
                                         start=(pr == 0),
                                         stop=(pr == KB // 2 - 1),
                                         perf_mode=DR)
                        nc.tensor.matmul(rs_ap(t, qg), lhsT=ones2_3, rhs=pt3,
                                         start=(pr == 0),
                                         stop=(pr == KB // 2 - 1),
                                         perf_mode=DR)

                # normalize: ht = ut * broadcast(1/rowsum)
                for ti, t in enumerate("AB"):
                    rv = bp.tile([1, QG], F32, tag=f"rv_{t}",
                                 name=f"rv_{t}_{qg}")
                    nc.vector.reciprocal_approx_fast(rv[:], rs_ap(t, qg))
                    rbc = bp.tile([128, QG], F32, tag=f"rbc_{t}",
                                  name=f"rbc_{t}_{qg}")
                    nc.gpsimd.partition_broadcast(rbc[:], rv[:])
                    nc.vector.tensor_tensor(ht[t][:, q_sl], ut_ps[t][:],
                                            rbc[:], op=mybir.AluOpType.mult)
                    h8 = bp.tile([128, QG], F8, tag=f"h8_{t}",
                                 name=f"h8_{t}_{qg}")
                    nc.gpsimd.tensor_scalar(h8[:], ht[t][:, q_sl],
                                            float(H_SCALE), None,
                                            op0=mybir.AluOpType.mult)
                    nc.sync.dma_start(
                        out=h_loc[qg * 128:(qg + 1) * 128,
                                  ti * QG:(ti + 1) * QG],
                        in_=h8[:])
                nc.gpsimd.collective_compute(
                    "AllGather", mybir.AluOpType.bypass,
                    replica_groups=[list(range(NCORES))],
                    ins=[h_loc[qg * 128:(qg + 1) * 128, :]],
                    outs=[h_sh[qg * 1024:(qg + 1) * 1024, :]])

            if dbg:
                for t in "AB":
                    nc.sync.dma_start(out=dbg_d[f"ht_{t}"][:], in_=ht[t][:])

            # ---------------- phase 2: dense count-matrix aggregation
            if stage >= 2:
                hTall = bp.tile([128, 2 * N], F8, tag="hTall",
                                name="hTall")
                for h in range(2):
                    for q in range(4):  # split over cores -> 4 queues
                        nc.sync.dma_start(
                            out=hTall[:, h * 8192 + q * 2048:
                                      h * 8192 + (q + 1) * 2048
                                      ].rearrange("f (c u) -> f c u", c=2),
                            in_=h_sh[h * 1024 + q * 256:
                                     h * 1024 + (q + 1) * 256, :
                                     ].rearrange("(c f) u -> f c u", f=128))

                def hT_blk(t, b):
                    # virtual src-block b -> hTall 128-col block for type t
                    hh, cc, ww = b >> 5, (b >> 2) & 7, b & 3
                    j = hh * 64 + cc * 8 + (0 if t == "A" else 4) + ww
                    return hTall[:, j * 128:(j + 1) * 128]

                po = {}
                for t in "AB":
                    po[t] = ps_big.tile([128, 1024], F32, tag="sc",
                                        name=f"po_{t}")

                def emit_fixups():
                    for t in "AB":
                        for h in range(2):
                            h_sl = slice(h * 512, (h + 1) * 512)
                            nc.tensor.matmul(po[t][:, h_sl],
                                             lhsT=W[f"wrT_{t}"],
                                             rhs=ht[t][:, h_sl],
                                             start=False, stop=False)
                            for part in ("h", "l"):
                                nc.tensor.matmul(po[t][:, h_sl],
                                                 lhsT=RW[f"c0{part}_{t}"],
                                                 rhs=ones_row[:],
                                                 start=False, stop=False)
                    for g in G:
                        for h in range(2):
                            for part in ("h", "l"):
                                nc.tensor.matmul(
                                    po[DST_T[g]][:, h * 512:(h + 1) * 512],
                                    lhsT=RW[f"c1{part}_{g}"],
                                    rhs=DEG[g][:, h * 512:(h + 1) * 512],
                                    start=False, stop=False)

                # table computation is software-pipelined one sbp ahead of
                # the aggregation matmuls so the fp8 copy latency hides;
                # tpB borrows the idle ps_rs slots.
                def emit_tab(i):
                    scg, sbp = divmod(i, 4)
                    j0 = scg * 8 + sbp * 2
                    tpA = ps_ut.tile([128, 512], F32, tag="ut",
                                     name=f"tpA_{scg}_{sbp}")
                    tpB = ps_rs.tile([128, 512], F32, tag="rs",
                                     name=f"tpB_{scg}_{sbp}")
                    for j in range(2):
                        nc.tensor.matmul(tpA[:, j * 256:(j + 1) * 256],
                                         lhsT=hT_blk("A", j0 + j),
                                         rhs=wl_cat_A,
                                         start=True, stop=True)
                        nc.tensor.matmul(tpB[:, j * 128:(j + 1) * 128],
                                         lhsT=hT_blk("B", j0 + j),
                                         rhs=W["wlT_BA"],
                                         start=True, stop=True)
                    tabA = tabp.tile([128, 512], F8, tag="tabA",
                                     name=f"tabA_{scg}_{sbp}")
                    nc.scalar.activation(tabA[:], tpA[:],
                                         mybir.ActivationFunctionType.Copy)
                    tabB = tabp.tile([128, 256], F8, tag="tabB",
                                     name=f"tabB_{scg}_{sbp}")
                    nc.vector.tensor_copy(tabB[:], tpB[:, :256])
                    tabA3 = tabA[:].rearrange("p (j f) -> p j f", j=2)
                    tabB3 = tabB[:].rearrange("p (j f) -> p j f", j=2)
                    return {"AB": tabA3[:, :, 0:128],
                            "AA": tabA3[:, :, 128:256],
                            "BA": tabB3}

                # ct chunks ride freed x0/kt/vt slots past the ctp pool.
                # kt/vt free only when attention ends, so their chunks go
                # last in consumption order; x0 frees right after QKV.
                spill = {12: ("x0t_A", bp), 13: ("x0t_B", bp),
                         20: ("kt_A", bp), 21: ("kt_B", bp),
                         22: ("vt_A", bp), 23: ("vt_B", bp)}
                tabs = {0: emit_tab(0)}
                started = set()
                for scg in range(8):
                    ct_s = {}
                    for g in G:
                        k = scg * 3 + GI[g]
                        if k in spill:
                            tag, pool = spill[k]
                            ct_s[g] = pool.tile([128, 8 * R], F8, tag=tag,
                                                name=f"ct_{g}_{scg}")
                        else:
                            ct_s[g] = ctp.tile([128, 8 * R], F8, tag="ct",
                                               name=f"ct_{g}_{scg}")
                        for q in range(4):  # split -> 4 parallel queues
                            nc.sync.dma_start(
                                out=ct_s[g][:, q * 2048:(q + 1) * 2048],
                                in_=ct[g][scg * 128:(scg + 1) * 128,
                                          q * 2048:(q + 1) * 2048])
                    for sbp in range(4):
                        i = scg * 4 + sbp
                        if i + 1 < 32:
                            tabs[i + 1] = emit_tab(i + 1)
                        lhsT_of = tabs.pop(i)
                        last = (scg == 7 and sbp == 3)
                        for g in G:
                            for h in range(2):
                                rhs = ct_s[g][:, sbp * 2048 + h * 1024:
                                              sbp * 2048 + (h + 1) * 1024
                                              ].rearrange(
                                                  "p (j d) -> p j d", j=2)
                                is_stop = last and (
                                    g == ("AB" if DST_T[g] == "B" else "AA"))
                                key = (DST_T[g], h)
                                nc.tensor.matmul(
                                    po[DST_T[g]][:, h * 512:(h + 1) * 512],
                                    lhsT=lhsT_of[g], rhs=rhs,
                                    start=(key not in started),
                                    stop=is_stop, perf_mode=DR)
                                started.add(key)
                    if scg == 0:
                        emit_fixups()

                for t in "AB":
                    for h in range(2):
                        osb = bp.tile([128, 512], F32, tag=f"osb_{t}{h}",
                                      name=f"osb_{t}_{h}")
                        nc.vector.tensor_copy(osb[:],
                                              po[t][:, h * 512:(h + 1) * 512])
                        nc.sync.dma_start(
                            out=out_d[t][:, h * 512:(h + 1) * 512],
                            in_=osb[:])

    nc.compile()
    return nc


# ---------------------------------------------------------------- host prep

def _pos_of_src():
    """global node id -> ct row position (virtual block * 128 + s)."""
    src = np.arange(N)
    c = src >> 10
    rr = src & 1023
    half = rr >> 9
    w = rr & 511
    wsub = w >> 7
    s = w & 127
    return (half * 32 + c * 4 + wsub) * 128 + s


def _hl(x):
    """split f32 vector into bf16 (hi, lo)."""
    hi = x.astype(BF16)
    lo = (x - hi.astype(np.float32)).astype(BF16)
    return hi, lo


def _prep(inputs, dbg=False):
    ins = {k: np.asarray(v) for k, v in inputs.items()}

    def bf(x):
        return np.ascontiguousarray(np.asarray(x, np.float32)).astype(BF16)

    com = {}
    for t in "AB":
        iw = ins[f"inW_{t}"].astype(np.float32)
        ib = ins[f"inB_{t}"].astype(np.float32)
        ow = ins[f"outW_{t}"].astype(np.float32)
        ob = ins[f"outB_{t}"].astype(np.float32)
        com[f"wqT_{t}"] = iw[0:128].T
        com[f"wkT_{t}"] = iw[128:256].T
        com[f"wvoT_{t}"] = (ow @ iw[256:384]).T
        com[f"bqs_{t}"] = ib[0:128] * SCALE
        com[f"bout_eff_{t}"] = ow @ ib[256:384] + ob
    c1 = {}
    for g in G:
        com[f"wlT_{g}"] = ins[f"wl_{g}"].astype(np.float32).T * WL_SCALE
        c1[g] = (ins[f"wl_{g}"].astype(np.float32)
                 @ com[f"bout_eff_{SRC_T[g]}"])
    com["wrT_B"] = ins["wr_AB"].astype(np.float32).T
    com["wrT_A"] = (ins["wr_BA"] + ins["wr_AA"]).astype(np.float32).T
    c0 = {
        "B": (ins["bl_AB"].astype(np.float32)
              + ins["wr_AB"].astype(np.float32) @ com["bout_eff_B"]),
        "A": (ins["bl_BA"].astype(np.float32)
              + ins["bl_AA"].astype(np.float32)
              + (ins["wr_BA"] + ins["wr_AA"]).astype(np.float32)
              @ com["bout_eff_A"]),
    }
    rbc = {}
    for t in "AB":
        rbc[f"c0h_{t}"], rbc[f"c0l_{t}"] = _hl(c0[t])
    for g in G:
        rbc[f"c1h_{g}"], rbc[f"c1l_{g}"] = _hl(c1[g])

    wblob = bf(np.concatenate([com[k] for k in WB_ORDER], axis=1))
    cblob = np.stack([com[f"bqs_{t}"] for t in "AB"],
                     axis=1).astype(np.float32)

    x0T = {t: np.ascontiguousarray(
        ins[f"x_{t}"][:, 0, :].astype(np.float32).T).astype(FP8)
        for t in "AB"}

    pos_of = _pos_of_src()
    cts = {}
    degs = {}
    for g in G:
        src = np.asarray(ins[f"ei_{g}"][0], np.int64)
        dst = np.asarray(ins[f"ei_{g}"][1], np.int64)
        per_core = []
        dgs = []
        for c in range(NCORES):
            sel = (dst >> 10) == c
            s_c = pos_of[src[sel]]
            d_c = dst[sel] - c * R
            cmat = np.zeros((N, R), np.float32)
            np.add.at(cmat, (s_c, d_c), 1.0)
            cmat *= 1.0 / TAB_SCALE
            swz = np.ascontiguousarray(
                cmat.reshape(8, 4, 2, 128, 2, 512).transpose(0, 3, 1, 4, 2, 5)
                .reshape(1024, 8 * R))
            per_core.append(swz.astype(FP8))
            dgs.append(np.bincount(d_c, minlength=R).astype(BF16))
        cts[g] = per_core
        degs[g] = dgs

    in_maps = []
    for c in range(NCORES):
        rblob = np.concatenate(
            [rbc[k].astype(BF16) for k in RB_ORDER]
            + [degs[g][c] for g in G]).reshape(1, -1)
        m = {"wblob": wblob, "cblob": cblob, "rblob": rblob}
        for t in "AB":
            m[f"x0t_{t}"] = x0T[t]
            m[f"x0q_{t}"] = np.ascontiguousarray(x0T[t][:, c * R:(c + 1) * R])
        for g in G:
            m[f"ct_{g}"] = cts[g][c]
        in_maps.append(m)
    return in_maps


def kernel(**inputs):
    in_maps = _prep(inputs)
    if "prog" not in _PROG_CACHE:
        _PROG_CACHE["prog"] = build_program()
    nc = _PROG_CACHE["prog"]
    res = bass_utils.run_bass_kernel_spmd(
        nc, in_maps, core_ids=list(range(NCORES)))
    x_A = np.asarray(inputs["x_A"], np.float32)
    x_B = np.asarray(inputs["x_B"], np.float32)
    new_A = x_A.copy()
    new_B = x_B.copy()
    for c in range(NCORES):
        new_A[c * R:(c + 1) * R, 0, :] = res.results[c]["out_A"].T
        new_B[c * R:(c + 1) * R, 0, :] = res.results[c]["out_B"].T
    return new_A, new_B
